# revision 12
# baseline (speedup 1.0000x reference)
"""CrossNet layer (encoder Dense + 4 cross layers) on 8 trn2 NeuronCores.

Pure data parallelism: batch 1024 is split into 8 shards of 128 rows;
encoder weights + tiny cross weights are replicated per core.

Math: with h = x @ W_enc + b_enc, x0 = h, the cross recurrence
    x_{l+1} = x_l + x0 * (x_l @ w_l) + b_l
keeps the closed form x_l = x0 * c_l + B_l with per-row scalar c_l and
H-vector B_l = sum_{j<l} b_j, since
    s_l = x_l @ w_l = c_l * (x0 @ w_l) + B_l @ w_l = c_l * p_l + q_l
    c_{l+1} = c_l * (1 + p_l) + q_l,   c_0 = 1.
So the device only needs the big matmul h, P = x0 @ Wc (Wc = ws^T),
the 4x4 table Q[j,l] = b_j @ w_l (q_l = sum_{j<l} Q[j,l]), a 4-step scan
for c, and out = x0 * c_4 + B_4.

Schedule: W streams in 16 [128,512] chunks over SWDGE (completes roughly
in issue order), n-outer matmul loop so the first H-half's transpose/P
work and output DMA overlap the second half's matmuls.
"""

import numpy as np

B, D, H, DEPTH = 1024, 1024, 1024, 4
N_CORES = 8
BS = B // N_CORES  # batch rows per core
KT = D // 128      # contraction k-tiles
NT = H // 512      # psum n-tiles

_cache = {}


def _patch_tile_drain(max_waits: int = 1):
    """walrus in this image allows only 1 sync-wait per instruction; the stock
    Tile end-of-kernel drain carries the whole global clock on one SP Drain and
    codegen fails. Split the waits across a chain of SP nops instead."""
    import concourse.tile as tile
    from concourse.vector_clock import ScopedClock
    from concourse import mybir

    if getattr(tile.TileContext, "_drain_patched", False):
        return

    def _drain_and_barrier(self, tick_clock, wait_clock):
        nc = self.nc
        carrier = nc.sync.nop()
        wait_clock.add_sem_waits(
            carrier.ins, ScopedClock({None: tick_clock.global_clock})
        )
        si = carrier.ins.sync_info
        if si is not None and si.on_wait and len(si.on_wait) > max_waits:
            waits = list(si.on_wait)
            carrier.ins.sync_info = mybir.SyncInfo(
                on_wait=waits[:max_waits], on_update=list(si.on_update or [])
            )
            rest = waits[max_waits:]
            while rest:
                extra = nc.sync.nop()
                extra.ins.sync_info = mybir.SyncInfo(
                    on_wait=rest[:max_waits], on_update=[]
                )
                rest = rest[max_waits:]
        nc.sync.drain()

        nc.all_engine_barrier()
        assert self.sems is not None
        popped = nc._tile_sem_poison_stack.pop()
        assert popped is self._sem_poison
        nc.clear_and_free_semaphores(list(self.sems.allocated().values()))

    tile.TileContext._drain_and_barrier = _drain_and_barrier
    tile.TileContext._drain_patched = True


def _split_multi_waits(nc):
    """walrus here allows only one sync-wait per instruction: move extra waits
    onto same-engine NoOps inserted immediately before the instruction."""
    from concourse import mybir

    for fn in nc.m.functions:
        for bb in fn.blocks:
            out = []
            for inst in bb.instructions:
                si = inst.sync_info
                if si is not None and si.on_wait and len(si.on_wait) > 1:
                    waits = list(si.on_wait)
                    for i, w in enumerate(waits[:-1]):
                        nop = mybir.InstNoOp(name=f"{inst.name}-w{i}", ins=[], outs=[])
                        nop.engine = inst.engine
                        nop.sync_info = mybir.SyncInfo(on_wait=[w], on_update=[])
                        out.append(nop)
                    inst.sync_info = mybir.SyncInfo(
                        on_wait=[waits[-1]], on_update=list(si.on_update or [])
                    )
                out.append(inst)
            bb.instructions[:] = out


def _build(use_f32r=True, split=True):
    from contextlib import ExitStack

    import concourse.bass as bass
    import concourse.tile as tile
    from concourse import mybir

    _patch_tile_drain()

    fp32 = mybir.dt.float32
    f32r = mybir.dt.float32r
    i32 = mybir.dt.int32
    Alu = mybir.AluOpType

    nc = bass.Bass()
    x_in = nc.declare_dram_parameter("x", [BS, D], fp32, isOutput=False)
    w_in = nc.declare_dram_parameter("w", [D, H], fp32, isOutput=False)
    be_in = nc.declare_dram_parameter("be", [1, H], fp32, isOutput=False)
    ws_in = nc.declare_dram_parameter("ws", [DEPTH, H], fp32, isOutput=False)
    bs_in = nc.declare_dram_parameter("bs", [DEPTH, H], fp32, isOutput=False)
    y_out = nc.declare_dram_parameter("y", [BS, H], fp32, isOutput=True)

    with ExitStack() as ctx:
        tc = ctx.enter_context(tile.TileContext(nc))
        cpool = ctx.enter_context(tc.tile_pool(name="const", bufs=1))
        wpool = ctx.enter_context(tc.tile_pool(name="w", bufs=2 * KT))
        iop = ctx.enter_context(tc.tile_pool(name="io", bufs=1))
        xtp = ctx.enter_context(tc.tile_pool(name="xt", bufs=KT))
        htp = ctx.enter_context(tc.tile_pool(name="ht", bufs=KT))
        smp = ctx.enter_context(tc.tile_pool(name="sm", bufs=KT))
        pst = ctx.enter_context(tc.tile_pool(name="pst", bufs=2, space="PSUM"))
        psh = ctx.enter_context(tc.tile_pool(name="psh", bufs=2, space="PSUM"))
        psb = ctx.enter_context(tc.tile_pool(name="psb", bufs=2, space="PSUM"))
        psq = ctx.enter_context(tc.tile_pool(name="psq", bufs=1, space="PSUM"))

        # ---- input DMAs -------------------------------------------------
        # x + small tensors on HWDGE; W chunks stream on SWDGE so they
        # complete roughly in issue order (n-half 0 first).
        x_sb = iop.tile([BS, D], fp32)
        nc.sync.dma_start(x_sb[:], x_in[:])
        be_f = iop.tile([1, H], fp32)
        nc.sync.dma_start(be_f[:], be_in[:])
        be_sb = iop.tile([1, H], f32r if use_f32r else fp32)
        nc.vector.tensor_copy(be_sb[:], be_f[:])
        ws_sb = iop.tile([DEPTH, H], fp32)
        nc.sync.dma_start(ws_sb[:], ws_in[:])
        bs_sb = iop.tile([DEPTH, H], fp32)
        nc.sync.dma_start(bs_sb[:], bs_in[:])
        w_sb = []      # fp32 staging tiles
        w_r = []       # rounded f32r tiles the matmul reads
        w_dmas = []
        for k in range(KT):
            wk = wpool.tile([128, H], fp32, tag="wk", name=f"w{k}")
            w_dmas.append(nc.sync.dma_start(wk[:], w_in[k * 128 : (k + 1) * 128, :]))
            w_sb.append(wk)
            wrk = wpool.tile([128, H], f32r if use_f32r else fp32, tag="wr", name=f"wr{k}")
            nc.vector.tensor_copy(wrk[:], wk[:])
            w_r.append(wrk)

        # ---- constants --------------------------------------------------
        ident = cpool.tile([128, 128], fp32)
        row_i = cpool.tile([128, 128], i32)
        col_i = cpool.tile([128, 128], i32)
        nc.gpsimd.iota(row_i[:], pattern=[[0, 128]], base=0, channel_multiplier=1)
        nc.gpsimd.iota(col_i[:], pattern=[[1, 128]], base=0, channel_multiplier=0)
        nc.vector.tensor_tensor(ident[:], row_i[:], col_i[:], Alu.is_equal)

        ones1 = cpool.tile([1, 128], fp32)
        nc.gpsimd.memset(ones1[:], 1.0)
        ones1r = cpool.tile([1, 128], f32r if use_f32r else fp32)
        nc.vector.tensor_copy(ones1r[:], ones1[:])  # memset can't write f32r
        ones4 = cpool.tile([4, 128], fp32)
        nc.gpsimd.memset(ones4[:], 1.0)
        ones4r = cpool.tile([4, 128], f32r if use_f32r else fp32)
        nc.vector.tensor_copy(ones4r[:], ones4[:])
        maskL = cpool.tile([4, 4], fp32)  # maskL[j,l] = 1 if j < l
        nc.vector.tensor_tensor(maskL[:], row_i[0:4, 0:4], col_i[0:4, 0:4], Alu.is_lt)

        # ---- Wc/Bs^T tiles [128(h), 4] via PE transpose -----------------
        wc_sb, bst_sb = [], []
        for k in range(KT):
            tp = pst.tile([128, 128], fp32, tag="tp")
            nc.tensor.transpose(
                tp[:, 0:4], ws_sb[:, k * 128 : (k + 1) * 128], ident[0:4, 0:4]
            )
            wck = smp.tile([128, 4], fp32, tag="wc")
            nc.scalar.copy(wck[:], tp[:, 0:4])
            wc_sb.append(wck)
        for k in range(KT):
            tp = pst.tile([128, 128], fp32, tag="tp")
            nc.tensor.transpose(
                tp[:, 0:4], bs_sb[:, k * 128 : (k + 1) * 128], ident[0:4, 0:4]
            )
            bsk = smp.tile([128, 4], fp32, tag="bst")
            nc.scalar.copy(bsk[:], tp[:, 0:4])
            bst_sb.append(bsk)

        # ---- Q = Bs^T.T @ Wc -> q_l = sum_{j<l} Q[j,l] ------------------
        q_ps = psq.tile([4, 4], fp32, tag="q")
        for k in range(KT):
            nc.tensor.matmul(
                q_ps[:], bst_sb[k][:], wc_sb[k][:], start=(k == 0), stop=(k == KT - 1)
            )
        qm_sb = cpool.tile([4, 4], fp32)
        nc.vector.tensor_tensor(qm_sb[:], q_ps[:], maskL[:], Alu.mult)
        qrow_ps = psq.tile([1, 4], fp32, tag="q")
        nc.tensor.matmul(qrow_ps[:], ones4[:, 0:1], qm_sb[:], start=True, stop=True)
        qrow_sb = cpool.tile([1, 4], fp32)
        nc.scalar.copy(qrow_sb[:], qrow_ps[:])
        qb_ps = psq.tile([128, 4], fp32, tag="q")
        nc.tensor.matmul(qb_ps[:], ones1[:], qrow_sb[:], start=True, stop=True)

        # bs rounded for the f32r B4 broadcast matmuls (emitted post-k-loop)
        bs_r = iop.tile([DEPTH, H], f32r if use_f32r else fp32)
        nc.vector.tensor_copy(bs_r[:], bs_sb[:])

        # ---- x^T tiles via PE transpose ---------------------------------
        xt_sb = []
        for k in range(KT):
            tp = pst.tile([128, 128], fp32, tag="tp")
            nc.tensor.transpose(tp[:], x_sb[:, k * 128 : (k + 1) * 128], ident[:])
            xtk = xtp.tile([128, 128], f32r if use_f32r else fp32, tag="xt")
            nc.vector.tensor_copy(xtk[:], tp[:])
            xt_sb.append(xtk)

        # ---- big matmul h = x @ W + be (k-outer, n-inner) ---------------
        h_sb = iop.tile([BS, H], fp32)
        out_sb = iop.tile([BS, H], fp32)
        pt_ps = psq.tile([128, 4], fp32, tag="pt")
        c_sb = cpool.tile([128, 4], fp32)

        from concourse.tile_rust import add_dep_helper

        h_ps = [psh.tile([128, 512], fp32, tag="hps", name=f"hps{n}") for n in range(NT)]
        for n in range(NT):  # bias first: only needs be_sb, starts the group
            nc.tensor.matmul(
                h_ps[n][:], ones1r[:], be_sb[:, n * 512 : (n + 1) * 512],
                start=True, stop=False,
            )
        for k in range(KT):
            for n in range(NT):
                mm = nc.tensor.matmul(
                    h_ps[n][:], xt_sb[k][:], w_r[k][:, n * 512 : (n + 1) * 512],
                    start=False, stop=(k == KT - 1),
                )
                # stagger W streaming: chunk k+2 issues once chunk k is consumed,
                # so chunks arrive in order and PE is fed instead of waiting for
                # a 4MB round-robin to finish
                if n == 0 and k + 2 < KT:
                    add_dep_helper(w_dmas[k + 2].ins, mm.ins, reason="w-stream")
        for n in range(NT):
            nc.scalar.copy(h_sb[:, n * 512 : (n + 1) * 512], h_ps[n][:])

        # ---- B4 broadcast rows (f32r: cheap, off critical path) ---------
        b4_ps = []
        for n in range(NT):
            b4 = psb.tile([128, 512], fp32, tag="b4", name=f"b4ps{n}")
            nc.tensor.matmul(
                b4[:], ones4r[:], bs_r[:, n * 512 : (n + 1) * 512],
                start=True, stop=True,
            )
            b4_ps.append(b4)

        # ---- h^T tiles + P^T = x0 @ Wc ----------------------------------
        def transpose_pt(j):
            tp = pst.tile([128, 128], fp32, tag="tp", name=f"htp{j}")
            nc.tensor.transpose(tp[:], h_sb[:, j * 128 : (j + 1) * 128], ident[:])
            htj = htp.tile([128, 128], fp32, tag="ht", name=f"ht{j}")
            nc.scalar.copy(htj[:], tp[:])
            nc.tensor.matmul(
                pt_ps[:], htj[:], wc_sb[j][:],
                start=(j == 0), stop=(j == KT - 1),
                skip_group_check=True,
            )

        for j in range(KT):
            transpose_pt(j)

        # ---- c scan: c_{l+1} = (1 + P_l) * c_l + q_l --------------------
        at_sb = cpool.tile([128, 4], fp32)
        nc.vector.tensor_scalar_add(at_sb[:], pt_ps[:], 1.0)
        nc.vector.tensor_tensor_scan(
            c_sb[:], at_sb[:], qb_ps[:], 1.0, Alu.mult, Alu.add
        )

        # ---- final out = x0 * c4 + B4, per half, overlap DMA ------------
        for n in range(NT):
            nc.vector.scalar_tensor_tensor(
                out_sb[:, n * 512 : (n + 1) * 512],
                h_sb[:, n * 512 : (n + 1) * 512],
                c_sb[:, 3:4],
                b4_ps[n][:],
                Alu.mult,
                Alu.add,
            )
            nc.sync.dma_start(
                y_out[:, n * 512 : (n + 1) * 512], out_sb[:, n * 512 : (n + 1) * 512]
            )

    if split:
        _split_multi_waits(nc)
    return nc


def kernel(x, W_enc, b_enc, ws, bs):
    from concourse.bass_utils import run_bass_kernel_spmd

    if "nc" not in _cache:
        _cache["nc"] = _build()
    nc = _cache["nc"]

    x = np.ascontiguousarray(x, dtype=np.float32)
    in_maps = []
    for c in range(N_CORES):
        in_maps.append(
            {
                "x": x[c * BS : (c + 1) * BS],
                "w": np.ascontiguousarray(W_enc, dtype=np.float32),
                "be": np.ascontiguousarray(b_enc, dtype=np.float32).reshape(1, H),
                "ws": np.ascontiguousarray(ws, dtype=np.float32).reshape(DEPTH, H),
                "bs": np.ascontiguousarray(bs, dtype=np.float32).reshape(DEPTH, H),
            }
        )
    res = run_bass_kernel_spmd(nc, in_maps, list(range(N_CORES)))
    return np.concatenate([res.results[c]["y"] for c in range(N_CORES)], axis=0)


# revision 13
# speedup vs baseline: 1.2038x; 1.2038x over previous
"""CrossNet layer (encoder Dense + 4 cross layers) on 8 trn2 NeuronCores.

Pure data parallelism: batch 1024 is split into 8 shards of 128 rows;
encoder weights + tiny cross weights are replicated per core.

Math: with h = x @ W_enc + b_enc, x0 = h, the cross recurrence
    x_{l+1} = x_l + x0 * (x_l @ w_l) + b_l
keeps the closed form x_l = x0 * c_l + B_l with per-row scalar c_l and
H-vector B_l = sum_{j<l} b_j, since
    s_l = x_l @ w_l = c_l * (x0 @ w_l) + B_l @ w_l = c_l * p_l + q_l
    c_{l+1} = c_l * (1 + p_l) + q_l,   c_0 = 1.
So the device only needs the big matmul h, P = x0 @ Wc (Wc = ws^T),
the 4x4 table Q[j,l] = b_j @ w_l (q_l = sum_{j<l} Q[j,l]), a 4-step scan
for c, and out = x0 * c_4 + B_4.

Schedule: W streams in 16 [128,512] chunks over SWDGE (completes roughly
in issue order), n-outer matmul loop so the first H-half's transpose/P
work and output DMA overlap the second half's matmuls.
"""

import numpy as np

B, D, H, DEPTH = 1024, 1024, 1024, 4
N_CORES = 8
BS = B // N_CORES  # batch rows per core
KT = D // 128      # contraction k-tiles
NT = H // 512      # psum n-tiles

_cache = {}


def _patch_tile_drain(max_waits: int = 1):
    """walrus in this image allows only 1 sync-wait per instruction; the stock
    Tile end-of-kernel drain carries the whole global clock on one SP Drain and
    codegen fails. Split the waits across a chain of SP nops instead."""
    import concourse.tile as tile
    from concourse.vector_clock import ScopedClock
    from concourse import mybir

    if getattr(tile.TileContext, "_drain_patched", False):
        return

    def _drain_and_barrier(self, tick_clock, wait_clock):
        nc = self.nc
        carrier = nc.sync.nop()
        wait_clock.add_sem_waits(
            carrier.ins, ScopedClock({None: tick_clock.global_clock})
        )
        si = carrier.ins.sync_info
        if si is not None and si.on_wait and len(si.on_wait) > max_waits:
            waits = list(si.on_wait)
            carrier.ins.sync_info = mybir.SyncInfo(
                on_wait=waits[:max_waits], on_update=list(si.on_update or [])
            )
            rest = waits[max_waits:]
            while rest:
                extra = nc.sync.nop()
                extra.ins.sync_info = mybir.SyncInfo(
                    on_wait=rest[:max_waits], on_update=[]
                )
                rest = rest[max_waits:]
        nc.sync.drain()

        nc.all_engine_barrier()
        assert self.sems is not None
        popped = nc._tile_sem_poison_stack.pop()
        assert popped is self._sem_poison
        nc.clear_and_free_semaphores(list(self.sems.allocated().values()))

    tile.TileContext._drain_and_barrier = _drain_and_barrier
    tile.TileContext._drain_patched = True


def _split_multi_waits(nc):
    """walrus here allows only one sync-wait per instruction: move extra waits
    onto same-engine NoOps inserted immediately before the instruction."""
    from concourse import mybir

    for fn in nc.m.functions:
        for bb in fn.blocks:
            out = []
            for inst in bb.instructions:
                si = inst.sync_info
                if si is not None and si.on_wait and len(si.on_wait) > 1:
                    waits = list(si.on_wait)
                    for i, w in enumerate(waits[:-1]):
                        nop = mybir.InstNoOp(name=f"{inst.name}-w{i}", ins=[], outs=[])
                        nop.engine = inst.engine
                        nop.sync_info = mybir.SyncInfo(on_wait=[w], on_update=[])
                        out.append(nop)
                    inst.sync_info = mybir.SyncInfo(
                        on_wait=[waits[-1]], on_update=list(si.on_update or [])
                    )
                out.append(inst)
            bb.instructions[:] = out


def _build(use_f32r=True, split=True):
    from contextlib import ExitStack

    import concourse.bass as bass
    import concourse.tile as tile
    from concourse import mybir

    _patch_tile_drain()

    fp32 = mybir.dt.float32
    f32r = mybir.dt.float32r
    i32 = mybir.dt.int32
    Alu = mybir.AluOpType

    nc = bass.Bass()
    x_in = nc.declare_dram_parameter("x", [BS, D], fp32, isOutput=False)
    w_in = nc.declare_dram_parameter("w", [D, H], fp32, isOutput=False)
    be_in = nc.declare_dram_parameter("be", [1, H], fp32, isOutput=False)
    ws_in = nc.declare_dram_parameter("ws", [DEPTH, H], fp32, isOutput=False)
    bs_in = nc.declare_dram_parameter("bs", [DEPTH, H], fp32, isOutput=False)
    y_out = nc.declare_dram_parameter("y", [BS, H], fp32, isOutput=True)

    with ExitStack() as ctx:
        tc = ctx.enter_context(tile.TileContext(nc))
        cpool = ctx.enter_context(tc.tile_pool(name="const", bufs=1))
        wpool = ctx.enter_context(tc.tile_pool(name="w", bufs=2 * KT))
        iop = ctx.enter_context(tc.tile_pool(name="io", bufs=1))
        xtp = ctx.enter_context(tc.tile_pool(name="xt", bufs=KT))
        htp = ctx.enter_context(tc.tile_pool(name="ht", bufs=KT))
        smp = ctx.enter_context(tc.tile_pool(name="sm", bufs=KT))
        pst = ctx.enter_context(tc.tile_pool(name="pst", bufs=2, space="PSUM"))
        psh = ctx.enter_context(tc.tile_pool(name="psh", bufs=2, space="PSUM"))
        psb = ctx.enter_context(tc.tile_pool(name="psb", bufs=2, space="PSUM"))
        psq = ctx.enter_context(tc.tile_pool(name="psq", bufs=1, space="PSUM"))

        # ---- input DMAs -------------------------------------------------
        # x + small tensors on HWDGE; W chunks stream on SWDGE so they
        # complete roughly in issue order (n-half 0 first).
        x_sb = iop.tile([BS, D], fp32)
        nc.sync.dma_start(x_sb[:], x_in[:])
        be_sb = iop.tile([1, H], f32r if use_f32r else fp32)
        nc.sync.dma_start(be_sb[:], be_in[:].bitcast(f32r) if use_f32r else be_in[:])
        ws_sb = iop.tile([DEPTH, H], fp32)
        nc.sync.dma_start(ws_sb[:], ws_in[:])
        bs_sb = iop.tile([DEPTH, H], fp32)
        nc.sync.dma_start(bs_sb[:], bs_in[:])
        w_r = []   # f32r tiles, DMA'd via bitcast (PE truncates low mantissa)
        w_dmas = []
        for k in range(KT):
            wrk = wpool.tile([128, H], f32r if use_f32r else fp32, tag="wr", name=f"wr{k}")
            src_ap = w_in[k * 128 : (k + 1) * 128, :]
            if use_f32r:
                src_ap = src_ap.bitcast(f32r)
            w_dmas.append(nc.sync.dma_start(wrk[:], src_ap))
            w_r.append(wrk)

        # ---- constants --------------------------------------------------
        ident = cpool.tile([128, 128], fp32)
        row_i = cpool.tile([128, 128], i32)
        col_i = cpool.tile([128, 128], i32)
        nc.gpsimd.iota(row_i[:], pattern=[[0, 128]], base=0, channel_multiplier=1)
        nc.gpsimd.iota(col_i[:], pattern=[[1, 128]], base=0, channel_multiplier=0)
        nc.vector.tensor_tensor(ident[:], row_i[:], col_i[:], Alu.is_equal)

        ones1 = cpool.tile([1, 128], fp32)
        nc.gpsimd.memset(ones1[:], 1.0)
        ones1r = cpool.tile([1, 128], f32r if use_f32r else fp32)
        nc.vector.tensor_copy(ones1r[:], ones1[:])  # memset can't write f32r
        ones4 = cpool.tile([4, 128], fp32)
        nc.gpsimd.memset(ones4[:], 1.0)
        ones4r = cpool.tile([4, 128], f32r if use_f32r else fp32)
        nc.vector.tensor_copy(ones4r[:], ones4[:])
        maskL = cpool.tile([4, 4], fp32)  # maskL[j,l] = 1 if j < l
        nc.vector.tensor_tensor(maskL[:], row_i[0:4, 0:4], col_i[0:4, 0:4], Alu.is_lt)

        # ---- Wc/Bs^T tiles [128(h), 4] via PE transpose -----------------
        wc_sb, bst_sb = [], []
        for k in range(KT):
            tp = pst.tile([128, 128], fp32, tag="tp")
            nc.tensor.transpose(
                tp[:, 0:4], ws_sb[:, k * 128 : (k + 1) * 128], ident[0:4, 0:4]
            )
            wck = smp.tile([128, 4], fp32, tag="wc")
            nc.scalar.copy(wck[:], tp[:, 0:4])
            wc_sb.append(wck)
        for k in range(KT):
            tp = pst.tile([128, 128], fp32, tag="tp")
            nc.tensor.transpose(
                tp[:, 0:4], bs_sb[:, k * 128 : (k + 1) * 128], ident[0:4, 0:4]
            )
            bsk = smp.tile([128, 4], fp32, tag="bst")
            nc.scalar.copy(bsk[:], tp[:, 0:4])
            bst_sb.append(bsk)

        # ---- Q = Bs^T.T @ Wc -> q_l = sum_{j<l} Q[j,l] ------------------
        q_ps = psq.tile([4, 4], fp32, tag="q")
        for k in range(KT):
            nc.tensor.matmul(
                q_ps[:], bst_sb[k][:], wc_sb[k][:], start=(k == 0), stop=(k == KT - 1)
            )
        qm_sb = cpool.tile([4, 4], fp32)
        nc.vector.tensor_tensor(qm_sb[:], q_ps[:], maskL[:], Alu.mult)
        qrow_ps = psq.tile([1, 4], fp32, tag="q")
        nc.tensor.matmul(qrow_ps[:], ones4[:, 0:1], qm_sb[:], start=True, stop=True)
        qrow_sb = cpool.tile([1, 4], fp32)
        nc.scalar.copy(qrow_sb[:], qrow_ps[:])
        qb_ps = psq.tile([128, 4], fp32, tag="q")
        nc.tensor.matmul(qb_ps[:], ones1[:], qrow_sb[:], start=True, stop=True)

        # bs rounded for the f32r B4 broadcast matmuls (emitted post-k-loop)
        bs_r = iop.tile([DEPTH, H], f32r if use_f32r else fp32)
        nc.vector.tensor_copy(bs_r[:], bs_sb[:])

        # ---- x^T tiles via PE transpose ---------------------------------
        xt_sb = []
        for k in range(KT):
            tp = pst.tile([128, 128], fp32, tag="tp")
            nc.tensor.transpose(tp[:], x_sb[:, k * 128 : (k + 1) * 128], ident[:])
            xtk = xtp.tile([128, 128], f32r if use_f32r else fp32, tag="xt")
            nc.vector.tensor_copy(xtk[:], tp[:])
            xt_sb.append(xtk)

        # ---- big matmul h = x @ W + be (k-outer, n-inner) ---------------
        h_sb = iop.tile([BS, H], fp32)
        out_sb = iop.tile([BS, H], fp32)
        pt_ps = psq.tile([128, 4], fp32, tag="pt")
        c_sb = cpool.tile([128, 4], fp32)

        from concourse.tile_rust import add_dep_helper

        h_ps = [psh.tile([128, 512], fp32, tag="hps", name=f"hps{n}") for n in range(NT)]
        for n in range(NT):  # bias first: only needs be_sb, starts the group
            nc.tensor.matmul(
                h_ps[n][:], ones1r[:], be_sb[:, n * 512 : (n + 1) * 512],
                start=True, stop=False,
            )
        for k in range(KT):
            for n in range(NT):
                mm = nc.tensor.matmul(
                    h_ps[n][:], xt_sb[k][:], w_r[k][:, n * 512 : (n + 1) * 512],
                    start=False, stop=(k == KT - 1),
                )
                # stagger W streaming: chunk k+2 issues once chunk k is consumed,
                # so chunks arrive in order and PE is fed instead of waiting for
                # a 4MB round-robin to finish
                if n == 0 and k + 4 < KT:
                    add_dep_helper(w_dmas[k + 4].ins, mm.ins, reason="w-stream")
        for n in range(NT):
            nc.scalar.copy(h_sb[:, n * 512 : (n + 1) * 512], h_ps[n][:])

        # ---- B4 broadcast rows (f32r: cheap, off critical path) ---------
        b4_ps = []
        for n in range(NT):
            b4 = psb.tile([128, 512], fp32, tag="b4", name=f"b4ps{n}")
            nc.tensor.matmul(
                b4[:], ones4r[:], bs_r[:, n * 512 : (n + 1) * 512],
                start=True, stop=True,
            )
            b4_ps.append(b4)

        # ---- h^T tiles + P^T = x0 @ Wc ----------------------------------
        def transpose_pt(j):
            tp = pst.tile([128, 128], fp32, tag="tp", name=f"htp{j}")
            nc.tensor.transpose(tp[:], h_sb[:, j * 128 : (j + 1) * 128], ident[:])
            htj = htp.tile([128, 128], fp32, tag="ht", name=f"ht{j}")
            nc.scalar.copy(htj[:], tp[:])
            nc.tensor.matmul(
                pt_ps[:], htj[:], wc_sb[j][:],
                start=(j == 0), stop=(j == KT - 1),
                skip_group_check=True,
            )

        for j in range(KT):
            transpose_pt(j)

        # ---- c scan: c_{l+1} = (1 + P_l) * c_l + q_l --------------------
        at_sb = cpool.tile([128, 4], fp32)
        nc.vector.tensor_scalar_add(at_sb[:], pt_ps[:], 1.0)
        nc.vector.tensor_tensor_scan(
            c_sb[:], at_sb[:], qb_ps[:], 1.0, Alu.mult, Alu.add
        )

        # ---- final out = x0 * c4 + B4, per half, overlap DMA ------------
        for n in range(NT):
            nc.vector.scalar_tensor_tensor(
                out_sb[:, n * 512 : (n + 1) * 512],
                h_sb[:, n * 512 : (n + 1) * 512],
                c_sb[:, 3:4],
                b4_ps[n][:],
                Alu.mult,
                Alu.add,
            )
            nc.sync.dma_start(
                y_out[:, n * 512 : (n + 1) * 512], out_sb[:, n * 512 : (n + 1) * 512]
            )

    if split:
        _split_multi_waits(nc)
    return nc


def kernel(x, W_enc, b_enc, ws, bs):
    from concourse.bass_utils import run_bass_kernel_spmd

    if "nc" not in _cache:
        _cache["nc"] = _build()
    nc = _cache["nc"]

    x = np.ascontiguousarray(x, dtype=np.float32)
    in_maps = []
    for c in range(N_CORES):
        in_maps.append(
            {
                "x": x[c * BS : (c + 1) * BS],
                "w": np.ascontiguousarray(W_enc, dtype=np.float32),
                "be": np.ascontiguousarray(b_enc, dtype=np.float32).reshape(1, H),
                "ws": np.ascontiguousarray(ws, dtype=np.float32).reshape(DEPTH, H),
                "bs": np.ascontiguousarray(bs, dtype=np.float32).reshape(DEPTH, H),
            }
        )
    res = run_bass_kernel_spmd(nc, in_maps, list(range(N_CORES)))
    return np.concatenate([res.results[c]["y"] for c in range(N_CORES)], axis=0)


# revision 18
# speedup vs baseline: 1.2079x; 1.0033x over previous
"""CrossNet layer (encoder Dense + 4 cross layers) on 8 trn2 NeuronCores.

Pure data parallelism: batch 1024 is split into 8 shards of 128 rows;
encoder weights + tiny cross weights are replicated per core.

Math: with h = x @ W_enc + b_enc, x0 = h, the cross recurrence
    x_{l+1} = x_l + x0 * (x_l @ w_l) + b_l
keeps the closed form x_l = x0 * c_l + B_l with per-row scalar c_l and
H-vector B_l = sum_{j<l} b_j, since
    s_l = x_l @ w_l = c_l * (x0 @ w_l) + B_l @ w_l = c_l * p_l + q_l
    c_{l+1} = c_l * (1 + p_l) + q_l,   c_0 = 1.
So the device only needs the big matmul h, P = x0 @ Wc (Wc = ws^T),
the 4x4 table Q[j,l] = b_j @ w_l (q_l = sum_{j<l} Q[j,l]), a 4-step scan
for c, and out = x0 * c_4 + B_4.

Schedule: W streams in 16 [128,512] chunks over SWDGE (completes roughly
in issue order), n-outer matmul loop so the first H-half's transpose/P
work and output DMA overlap the second half's matmuls.
"""

import numpy as np

B, D, H, DEPTH = 1024, 1024, 1024, 4
N_CORES = 8
BS = B // N_CORES  # batch rows per core
KT = D // 128      # contraction k-tiles
NT = H // 512      # psum n-tiles

_cache = {}


def _patch_tile_drain(max_waits: int = 1):
    """walrus in this image allows only 1 sync-wait per instruction; the stock
    Tile end-of-kernel drain carries the whole global clock on one SP Drain and
    codegen fails. Split the waits across a chain of SP nops instead."""
    import concourse.tile as tile
    from concourse.vector_clock import ScopedClock
    from concourse import mybir

    if getattr(tile.TileContext, "_drain_patched", False):
        return

    def _drain_and_barrier(self, tick_clock, wait_clock):
        nc = self.nc
        carrier = nc.sync.nop()
        wait_clock.add_sem_waits(
            carrier.ins, ScopedClock({None: tick_clock.global_clock})
        )
        si = carrier.ins.sync_info
        if si is not None and si.on_wait and len(si.on_wait) > max_waits:
            waits = list(si.on_wait)
            carrier.ins.sync_info = mybir.SyncInfo(
                on_wait=waits[:max_waits], on_update=list(si.on_update or [])
            )
            rest = waits[max_waits:]
            while rest:
                extra = nc.sync.nop()
                extra.ins.sync_info = mybir.SyncInfo(
                    on_wait=rest[:max_waits], on_update=[]
                )
                rest = rest[max_waits:]
        nc.sync.drain()

        nc.all_engine_barrier()
        assert self.sems is not None
        popped = nc._tile_sem_poison_stack.pop()
        assert popped is self._sem_poison
        nc.clear_and_free_semaphores(list(self.sems.allocated().values()))

    tile.TileContext._drain_and_barrier = _drain_and_barrier
    tile.TileContext._drain_patched = True


def _split_multi_waits(nc):
    """walrus here allows only one sync-wait per instruction: move extra waits
    onto same-engine NoOps inserted immediately before the instruction."""
    from concourse import mybir

    for fn in nc.m.functions:
        for bb in fn.blocks:
            out = []
            for inst in bb.instructions:
                si = inst.sync_info
                if si is not None and si.on_wait and len(si.on_wait) > 1:
                    waits = list(si.on_wait)
                    for i, w in enumerate(waits[:-1]):
                        nop = mybir.InstNoOp(name=f"{inst.name}-w{i}", ins=[], outs=[])
                        nop.engine = inst.engine
                        nop.sync_info = mybir.SyncInfo(on_wait=[w], on_update=[])
                        out.append(nop)
                    inst.sync_info = mybir.SyncInfo(
                        on_wait=[waits[-1]], on_update=list(si.on_update or [])
                    )
                out.append(inst)
            bb.instructions[:] = out


def _build(use_f32r=True, split=True):
    from contextlib import ExitStack

    import concourse.bass as bass
    import concourse.tile as tile
    from concourse import mybir

    _patch_tile_drain()

    fp32 = mybir.dt.float32
    f32r = mybir.dt.float32r
    i32 = mybir.dt.int32
    Alu = mybir.AluOpType

    nc = bass.Bass()
    x_in = nc.declare_dram_parameter("x", [BS, D], fp32, isOutput=False)
    w_in = nc.declare_dram_parameter("w", [D, H], fp32, isOutput=False)
    be_in = nc.declare_dram_parameter("be", [1, H], fp32, isOutput=False)
    ws_in = nc.declare_dram_parameter("ws", [DEPTH, H], fp32, isOutput=False)
    bs_in = nc.declare_dram_parameter("bs", [DEPTH, H], fp32, isOutput=False)
    y_out = nc.declare_dram_parameter("y", [BS, H], fp32, isOutput=True)

    with ExitStack() as ctx:
        tc = ctx.enter_context(tile.TileContext(nc))
        cpool = ctx.enter_context(tc.tile_pool(name="const", bufs=1))
        wpool = ctx.enter_context(tc.tile_pool(name="w", bufs=2 * KT))
        iop = ctx.enter_context(tc.tile_pool(name="io", bufs=1))
        xtp = ctx.enter_context(tc.tile_pool(name="xt", bufs=KT))
        htp = ctx.enter_context(tc.tile_pool(name="ht", bufs=KT))
        smp = ctx.enter_context(tc.tile_pool(name="sm", bufs=KT))
        pst = ctx.enter_context(tc.tile_pool(name="pst", bufs=2, space="PSUM"))
        psh = ctx.enter_context(tc.tile_pool(name="psh", bufs=2, space="PSUM"))
        psb = ctx.enter_context(tc.tile_pool(name="psb", bufs=2, space="PSUM"))
        psq = ctx.enter_context(tc.tile_pool(name="psq", bufs=1, space="PSUM"))

        # ---- input DMAs -------------------------------------------------
        # x + small tensors on HWDGE; W chunks stream on SWDGE so they
        # complete roughly in issue order (n-half 0 first).
        x_sb = iop.tile([BS, D], fp32)
        nc.sync.dma_start(x_sb[:], x_in[:])
        be_sb = iop.tile([1, H], f32r if use_f32r else fp32)
        nc.sync.dma_start(be_sb[:], be_in[:].bitcast(f32r) if use_f32r else be_in[:])
        ws_sb = iop.tile([DEPTH, H], fp32)
        nc.sync.dma_start(ws_sb[:], ws_in[:])
        bs_sb = iop.tile([DEPTH, H], fp32)
        nc.sync.dma_start(bs_sb[:], bs_in[:])
        from concourse.tile_rust import add_dep_helper

        w_r = []   # f32r tiles, DMA'd via bitcast (PE truncates low mantissa)
        w_dmas = []
        for k in range(KT):
            wrk = wpool.tile([128, H], f32r if use_f32r else fp32, tag="wr", name=f"wr{k}")
            src_ap = w_in[k * 128 : (k + 1) * 128, :]
            if use_f32r:
                src_ap = src_ap.bitcast(f32r)
            dma = nc.sync.dma_start(wrk[:], src_ap)
            if k >= 2:  # 2 chunks in flight -> near in-order arrival
                add_dep_helper(dma.ins, w_dmas[k - 2].ins, reason="w-chain")
            w_dmas.append(dma)
            w_r.append(wrk)

        # ---- constants --------------------------------------------------
        ident = cpool.tile([128, 128], fp32)
        row_i = cpool.tile([128, 128], i32)
        col_i = cpool.tile([128, 128], i32)
        nc.gpsimd.iota(row_i[:], pattern=[[0, 128]], base=0, channel_multiplier=1)
        nc.gpsimd.iota(col_i[:], pattern=[[1, 128]], base=0, channel_multiplier=0)
        nc.vector.tensor_tensor(ident[:], row_i[:], col_i[:], Alu.is_equal)

        ones1 = cpool.tile([1, 128], fp32)
        nc.gpsimd.memset(ones1[:], 1.0)
        ones1r = cpool.tile([1, 128], f32r if use_f32r else fp32)
        nc.vector.tensor_copy(ones1r[:], ones1[:])  # memset can't write f32r
        ones4 = cpool.tile([4, 128], fp32)
        nc.gpsimd.memset(ones4[:], 1.0)
        ones4r = cpool.tile([4, 128], f32r if use_f32r else fp32)
        nc.vector.tensor_copy(ones4r[:], ones4[:])
        maskL = cpool.tile([4, 4], fp32)  # maskL[j,l] = 1 if j < l
        nc.vector.tensor_tensor(maskL[:], row_i[0:4, 0:4], col_i[0:4, 0:4], Alu.is_lt)

        # ---- Wc/Bs^T tiles [128(h), 4] via PE transpose -----------------
        wc_sb, bst_sb = [], []
        for k in range(KT):
            tp = pst.tile([128, 128], fp32, tag="tp")
            nc.tensor.transpose(
                tp[:, 0:4], ws_sb[:, k * 128 : (k + 1) * 128], ident[0:4, 0:4]
            )
            wck = smp.tile([128, 4], fp32, tag="wc")
            nc.scalar.copy(wck[:], tp[:, 0:4])
            wc_sb.append(wck)
        for k in range(KT):
            tp = pst.tile([128, 128], fp32, tag="tp")
            nc.tensor.transpose(
                tp[:, 0:4], bs_sb[:, k * 128 : (k + 1) * 128], ident[0:4, 0:4]
            )
            bsk = smp.tile([128, 4], fp32, tag="bst")
            nc.scalar.copy(bsk[:], tp[:, 0:4])
            bst_sb.append(bsk)

        # ---- Q = Bs^T.T @ Wc -> q_l = sum_{j<l} Q[j,l] ------------------
        q_ps = psq.tile([4, 4], fp32, tag="q")
        for k in range(KT):
            nc.tensor.matmul(
                q_ps[:], bst_sb[k][:], wc_sb[k][:], start=(k == 0), stop=(k == KT - 1)
            )
        qm_sb = cpool.tile([4, 4], fp32)
        nc.vector.tensor_tensor(qm_sb[:], q_ps[:], maskL[:], Alu.mult)
        qrow_ps = psq.tile([1, 4], fp32, tag="q")
        nc.tensor.matmul(qrow_ps[:], ones4[:, 0:1], qm_sb[:], start=True, stop=True)
        qrow_sb = cpool.tile([1, 4], fp32)
        nc.scalar.copy(qrow_sb[:], qrow_ps[:])
        qb_ps = psq.tile([128, 4], fp32, tag="q")
        nc.tensor.matmul(qb_ps[:], ones1[:], qrow_sb[:], start=True, stop=True)

        # bs rounded for the f32r B4 broadcast matmuls (emitted post-k-loop)
        bs_r = iop.tile([DEPTH, H], f32r if use_f32r else fp32)
        nc.vector.tensor_copy(bs_r[:], bs_sb[:])

        # ---- x^T tiles via PE transpose ---------------------------------
        xt_sb = []
        for k in range(KT):
            tp = pst.tile([128, 128], fp32, tag="tp")
            nc.tensor.transpose(tp[:], x_sb[:, k * 128 : (k + 1) * 128], ident[:])
            xtk = xtp.tile([128, 128], f32r if use_f32r else fp32, tag="xt")
            nc.vector.tensor_copy(xtk[:], tp[:])
            xt_sb.append(xtk)

        # ---- big matmul h = x @ W + be (k-outer, n-inner) ---------------
        h_sb = iop.tile([BS, H], fp32)
        out_sb = iop.tile([BS, H], fp32)
        c_sb = cpool.tile([128, 4], fp32)

        h_ps = [psh.tile([128, 512], fp32, tag="hps", name=f"hps{n}") for n in range(NT)]
        for n in range(NT):  # bias first: only needs be_sb, starts the group
            nc.tensor.matmul(
                h_ps[n][:], ones1r[:], be_sb[:, n * 512 : (n + 1) * 512],
                start=True, stop=False,
            )
        for k in range(KT):
            for n in range(NT):
                nc.tensor.matmul(
                    h_ps[n][:], xt_sb[k][:], w_r[k][:, n * 512 : (n + 1) * 512],
                    start=False, stop=(k == KT - 1),
                )

        # ---- tail pipeline per 128-col tile: h copy -> h^T -> P matmul --
        # Pt[4,128] accumulates with the 4-column Wc as stationary operand
        # (LDWEIGHTS cost scales with stationary columns: ~free vs 128-col),
        # then one small transpose yields P^T[128,4]. Copies alternate
        # ACT/DVE so neither engine serializes the chain.
        pt4_ps = psq.tile([4, 128], fp32, tag="pt")
        for j in range(KT):
            n, c0 = j // 4, (j % 4) * 128
            if j % 2 == 0:
                nc.scalar.copy(
                    h_sb[:, j * 128 : (j + 1) * 128], h_ps[n][:, c0 : c0 + 128]
                )
            else:
                nc.vector.tensor_copy(
                    h_sb[:, j * 128 : (j + 1) * 128], h_ps[n][:, c0 : c0 + 128]
                )
            tp = pst.tile([128, 128], fp32, tag="tp", name=f"htp{j}")
            nc.tensor.transpose(tp[:], h_sb[:, j * 128 : (j + 1) * 128], ident[:])
            htj = htp.tile([128, 128], fp32, tag="ht", name=f"ht{j}")
            if j % 2 == 0:
                nc.vector.tensor_copy(htj[:], tp[:])
            else:
                nc.scalar.copy(htj[:], tp[:])
            nc.tensor.matmul(
                pt4_ps[:], wc_sb[j][:], htj[:],
                start=(j == 0), stop=(j == KT - 1),
                skip_group_check=True,
            )

        # ---- B4 broadcast rows (f32r: cheap) ----------------------------
        b4_ps = []
        for n in range(NT):
            b4 = psb.tile([128, 512], fp32, tag="b4", name=f"b4ps{n}")
            nc.tensor.matmul(
                b4[:], ones4r[:], bs_r[:, n * 512 : (n + 1) * 512],
                start=True, stop=True,
            )
            b4_ps.append(b4)

        pt4_sb = cpool.tile([4, 128], fp32)
        nc.scalar.copy(pt4_sb[:], pt4_ps[:])
        pt_ps = psq.tile([128, 4], fp32, tag="pt")
        nc.tensor.transpose(pt_ps[:], pt4_sb[:], ident[0:4, 0:4])

        # ---- c scan: c_{l+1} = (1 + P_l) * c_l + q_l --------------------
        at_sb = cpool.tile([128, 4], fp32)
        nc.vector.tensor_scalar_add(at_sb[:], pt_ps[:], 1.0)
        nc.vector.tensor_tensor_scan(
            c_sb[:], at_sb[:], qb_ps[:], 1.0, Alu.mult, Alu.add
        )

        # ---- final out = x0 * c4 + B4, per half, overlap DMA ------------
        for n in range(NT):
            nc.vector.scalar_tensor_tensor(
                out_sb[:, n * 512 : (n + 1) * 512],
                h_sb[:, n * 512 : (n + 1) * 512],
                c_sb[:, 3:4],
                b4_ps[n][:],
                Alu.mult,
                Alu.add,
            )
            nc.sync.dma_start(
                y_out[:, n * 512 : (n + 1) * 512], out_sb[:, n * 512 : (n + 1) * 512]
            )

    if split:
        _split_multi_waits(nc)
    return nc


def kernel(x, W_enc, b_enc, ws, bs):
    from concourse.bass_utils import run_bass_kernel_spmd

    if "nc" not in _cache:
        _cache["nc"] = _build()
    nc = _cache["nc"]

    x = np.ascontiguousarray(x, dtype=np.float32)
    in_maps = []
    for c in range(N_CORES):
        in_maps.append(
            {
                "x": x[c * BS : (c + 1) * BS],
                "w": np.ascontiguousarray(W_enc, dtype=np.float32),
                "be": np.ascontiguousarray(b_enc, dtype=np.float32).reshape(1, H),
                "ws": np.ascontiguousarray(ws, dtype=np.float32).reshape(DEPTH, H),
                "bs": np.ascontiguousarray(bs, dtype=np.float32).reshape(DEPTH, H),
            }
        )
    res = run_bass_kernel_spmd(nc, in_maps, list(range(N_CORES)))
    return np.concatenate([res.results[c]["y"] for c in range(N_CORES)], axis=0)
